# revision 33
# baseline (speedup 1.0000x reference)
"""Causal self-attention with sink, sharded over 8 TRN2 NeuronCores.

Sharding: batch x head-group. Core c handles batch b=c//4 and heads
[4*(c%4), 4*(c%4)+4). Each core computes its QKV projection slice,
attention for its 4 heads, and a partial output projection; the host sums
the 4 partials per batch.

v2 layout (all matmuls bf16, fp32 PSUM accumulation):
  - xT   [C=1024, T=2048]   (host pre-transposed x[b], bf16)
  - qT/kT in SBUF as head-pair tiles [128, T] (2 heads x 64 stacked)
  - v1   [128, 16, 4, 65]   v natural [t, d] per tk-chunk/head + ones col
                            (65th) that accumulates the softmax denominator
                            inside the PV matmul
  - S^T = K^T Q for BOTH heads of a pair per (tq-block, tk-chunk) into one
    2-bank PSUM tile [128, 1024]; the two matmuls land on PE row-tiles
    (0,0)/(64,0) and run concurrently. One exp over the 1024-wide tile.
  - PV:  out^T[d, tq] (+ denom row) accumulated in PSUM per head over
    tk-chunks; sink enters the denominator via a rank-1 matmul.
  - normalize via reciprocal_approx_fast + gpsimd partition_broadcast + mul
  - out projection produces natural [t, co] partials via yT-as-stationary
  - program order interleaves v/q23/k23 projections into pair-0 attention
    and the output projection into pair-1 attention so TensorE has dense
    work while ScalarE (exp) catches up, and vice versa.
"""

import os
import sys

import numpy as np

B, T, C = 2, 2048, 1024
H, D = 16, 64
NCORES = 8
HLOC = 4           # heads per core
GQ = HLOC * D      # 256 per-core q (or k or v) features
F = 3 * GQ         # 768 per-core qkv features
NCC = C // 128     # 8 contraction chunks
NTQ = T // 512     # 4 query blocks
NTK = T // 128     # 16 key chunks
SCALE = 1.0 / np.sqrt(D)

_BASS_PATHS = ("/opt/trn_rl_repo", "/root/.axon_site/_ro/trn_rl_repo")


def _import_bass():
    for p in _BASS_PATHS:
        if os.path.isdir(p) and p not in sys.path:
            sys.path.insert(0, p)
    import concourse.bass as bass
    import concourse.mybir as mybir
    import concourse.tile as tile
    from concourse import bacc
    return bass, mybir, tile, bacc


def build_nc(mm_dt="bfloat16", with_bias_qkv=True, with_bias_proj=True,
             exp_merge=True):
    """Build the per-core Bass program (same program for all 8 cores)."""
    bass, mybir, tile, bacc = _import_bass()
    f32 = mybir.dt.float32
    mdt = getattr(mybir.dt, mm_dt)
    AF = mybir.ActivationFunctionType

    nc = bacc.Bacc("TRN2", target_bir_lowering=False, debug=False)

    xT = nc.dram_tensor("xT", [C, T], mdt, kind="ExternalInput")
    wqkvT = nc.dram_tensor("wqkvT", [C, F], mdt, kind="ExternalInput")
    bqkv = nc.dram_tensor("bqkv", [1, F], mdt, kind="ExternalInput")
    wpT = nc.dram_tensor("wpT", [GQ, C], mdt, kind="ExternalInput")
    bp = nc.dram_tensor("bp", [1, C], mdt, kind="ExternalInput")
    sinkrow = nc.dram_tensor("sinkrow", [1, HLOC * 512], mdt, kind="ExternalInput")
    e65 = nc.dram_tensor("e65", [1, 65], mdt, kind="ExternalInput")
    out = nc.dram_tensor("out", [T, C], f32, kind="ExternalOutput")

    with tile.TileContext(nc) as tc:
        with (
            tc.tile_pool(name="const", bufs=1) as const,
            tc.tile_pool(name="persist", bufs=1) as persist,
            tc.tile_pool(name="xw", bufs=1) as xw,
            tc.tile_pool(name="projps", bufs=1, space="PSUM") as projps,
            tc.tile_pool(name="s2p", bufs=2, space="PSUM") as s2p,
            tc.tile_pool(name="pvp", bufs=3, space="PSUM") as pvp,
            tc.tile_pool(name="ep", bufs=6) as ep,
            tc.tile_pool(name="rp", bufs=8) as rp,
            tc.tile_pool(name="ostage", bufs=2) as ost,
        ):
            # ---- constants ----------------------------------------------
            ones_f = const.tile([1, 512], f32, tag="ones_f")
            nc.vector.memset(ones_f, 1.0)
            ones_r = const.tile([1, 512], mdt, tag="ones")
            nc.vector.tensor_copy(out=ones_r[:], in_=ones_f[:])
            ones_col = const.tile([128, 1], f32, tag="ones_col")
            nc.vector.memset(ones_col, 1.0)
            e65_r = const.tile([1, 65], mdt, tag="e65")
            bqkv_r = const.tile([1, F], mdt, tag="bqkv")
            bp_r = const.tile([1, C], mdt, tag="bp")
            sink_r = const.tile([1, HLOC * 512], mdt, tag="sink")
            # additive causal mask for the 128x128 diagonal blocks of S^T
            # (tk on partitions, tq on free): keep where tq >= tk.
            tri = const.tile([128, 128], f32, tag="tri")
            nc.gpsimd.memset(tri, 0.0)
            nc.gpsimd.affine_select(
                out=tri, in_=tri,
                compare_op=mybir.AluOpType.is_ge,
                fill=-1e30,
                base=0,
                pattern=[[1, 128]],
                channel_multiplier=-1,
            )

            # ---- persistent activations ---------------------------------
            # qk feature-block tiles: [q01, q23, k01, k23] each [128, T]
            qk = [persist.tile([128, T], mdt, tag=f"qk{i}", name=f"qk{i}") for i in range(4)]
            # v natural + ones column
            v1 = persist.tile([128, NTK, HLOC, 65], mdt, tag="v1", name="v1")
            nc.vector.tensor_copy(
                out=v1[:, :, :, 64:65],
                in_=ones_col[:, :].to_broadcast([128, NTK, HLOC, 1]),
            )
            # normalized attention output, head pairs stacked: yT[hp] [128, T]
            yT = [persist.tile([128, T], mdt, tag=f"yT{i}", name=f"yT{i}") for i in range(2)]

            # ---- weight/x loads -----------------------------------------
            # input loads: ~0.65us serial issue cost per dma_start on the
            # Sync queue; order so the first projection group's accumulation
            # chain (cc=0..7 over x block 0) unblocks as early as possible
            wq = [xw.tile([128, F], mdt, tag=f"wqkv{i}", name=f"wqkv{i}")
                  for i in range(NCC)]
            xt = [xw.tile([128, T], mdt, tag=f"xt{i}", name=f"xt{i}")
                  for i in range(NCC)]
            for i in range(NCC):
                nc.sync.dma_start(out=wq[i][:],
                                  in_=wqkvT[128 * i:128 * (i + 1), :])
                nc.sync.dma_start(
                    out=xt[i][:, 0:512], in_=xT[128 * i:128 * (i + 1), 0:512])
            # round-0 attention constants next, then the remaining x blocks
            nc.sync.dma_start(out=e65_r[:], in_=e65[:, :])
            nc.sync.dma_start(out=sink_r[:], in_=sinkrow[:, :])
            if with_bias_qkv:
                nc.sync.dma_start(out=bqkv_r[:], in_=bqkv[:, :])
            if with_bias_proj:
                nc.sync.dma_start(out=bp_r[:], in_=bp[:, :])
            for tqi in range(1, NTQ):
                for i in range(NCC):
                    nc.sync.dma_start(
                        out=xt[i][:, 512 * tqi:512 * (tqi + 1)],
                        in_=xT[128 * i:128 * (i + 1), 512 * tqi:512 * (tqi + 1)])
            wp = []
            for i in range(GQ // 128):
                t = xw.tile([128, C], mdt, tag=f"wp{i}", name=f"wp{i}")
                nc.sync.dma_start(out=t[:], in_=wpT[128 * i:128 * (i + 1), :])
                wp.append(t)

            # ---- emission helpers ---------------------------------------
            def proj_qk(fb, tqi):
                # feature blocks in wqkvT cols: q:[0,256) k:[256,512)
                # fb: 0=q01, 1=q23, 2=k01, 3=k23 -> qk[fb] directly
                col0 = [0, 128, 256, 384][fb]
                ps = projps.tile([128, 512], f32, tag="ps", name="ps")
                for cc in range(NCC):
                    nc.tensor.matmul(
                        ps[:, :],
                        wq[cc][:, col0:col0 + 128],
                        xt[cc][:, 512 * tqi:512 * (tqi + 1)],
                        start=(cc == 0),
                        stop=(cc == NCC - 1 and not with_bias_qkv),
                    )
                if with_bias_qkv:
                    nc.tensor.matmul(
                        ps[:, :],
                        bqkv_r[:, col0:col0 + 128],
                        ones_r[:, 0:512],
                        start=False, stop=True,
                    )
                nc.vector.tensor_copy(
                    out=qk[fb][:, 512 * tqi:512 * (tqi + 1)], in_=ps[:, :]
                )

            def proj_v(tb):
                ps = projps.tile([128, GQ], f32, tag="ps", name="psv")
                for cc in range(NCC):
                    nc.tensor.matmul(
                        ps[:, :],
                        xt[cc][:, 128 * tb:128 * (tb + 1)],
                        wq[cc][:, 512:768],
                        start=(cc == 0),
                        stop=(cc == NCC - 1 and not with_bias_qkv),
                    )
                if with_bias_qkv:
                    nc.tensor.matmul(
                        ps[:, :],
                        ones_r[:, 0:128],
                        bqkv_r[:, 512:768],
                        start=False, stop=True,
                    )
                nc.vector.tensor_copy(
                    out=v1[:, tb, :, 0:64],
                    in_=ps[:].rearrange("p (h d) -> p h d", h=HLOC),
                )

            def attention(hp, tqi, filler=None):
                # both heads of pair hp for query block tqi; `filler` is a
                # list of closures emitting independent TensorE work, drained
                # between chunk units so the PE never starves on exp latency.
                tq0 = 512 * tqi
                q_t, k_t = qk[hp], qk[2 + hp]
                pvs = None
                nchunks = tq0 // 128 + 4
                for tki in range(nchunks):
                    tk0 = 128 * tki
                    last = tki == nchunks - 1
                    full = tk0 < tq0
                    m = 0 if full else (tk0 - tq0) // 128
                    w = 512 - 128 * m
                    s2 = s2p.tile([128, 1024], f32, tag="s2", name="s2")
                    # S^T for both heads; stationary k at partition bases 0/64
                    # -> PE row-tiles (0,0)/(64,0), concurrent execution.
                    for j in range(2):
                        pb = 64 * j
                        nc.tensor.matmul(
                            s2[:, 512 * j:512 * j + w],
                            k_t[pb:pb + 64, tk0:tk0 + 128],
                            q_t[pb:pb + 64, tq0 + 128 * m:tq0 + 512],
                            start=True, stop=True,
                        )
                    if pvs is None:
                        # sink -> denominator row (also zero-fills rows
                        # 0..63). Emitted after the first S-pair so a wait
                        # on the pv bank (prev pair's normalize) doesn't
                        # head-block the ready S work in the in-order queue.
                        pvs = []
                        for j in range(2):
                            h = 2 * hp + j
                            pv = pvp.tile([65, 512], f32, tag="pv", name="pv")
                            nc.tensor.matmul(
                                pv[:, :], e65_r[:, :],
                                sink_r[0:1, h * 512:(h + 1) * 512],
                                start=True, stop=False,
                            )
                            pvs.append(pv)
                    e = ep.tile([128, 1024], mdt, tag="e", name="e")
                    if not exp_merge:
                        for j in range(2):
                            nc.scalar.activation(
                                out=e[:, 512 * j:512 * j + w],
                                in_=s2[:, 512 * j:512 * j + w],
                                func=AF.Exp, scale=SCALE)
                    elif full:
                        nc.scalar.activation(out=e[:, :], in_=s2[:, :],
                                             func=AF.Exp, scale=SCALE)
                    else:
                        # single ACT over the valid strided region of both heads
                        nc.scalar.activation(
                            out=e.rearrange("p (j q) -> p j q", j=2)[:, :, 0:w],
                            in_=s2.rearrange("p (j q) -> p j q", j=2)[:, :, 0:w],
                            func=AF.Exp, scale=SCALE)
                    if not full:
                        # causal mask: zero the upper triangle of the first
                        # visible 128 cols of e for both heads (idle GpSimd;
                        # keeps the DVE off the exp->PV critical path)
                        ev = e.rearrange("p (j q) -> p j q", j=2)[:, :, 0:128]
                        nc.gpsimd.affine_select(
                            out=ev, in_=ev,
                            compare_op=mybir.AluOpType.is_ge,
                            fill=0.0,
                            base=0,
                            pattern=[[0, 2], [1, 128]],
                            channel_multiplier=-1,
                        )
                    for j in range(2):
                        h = 2 * hp + j
                        nc.tensor.matmul(
                            pvs[j][:, 128 * m:512],
                            v1[:, tki, h, :],
                            e[:, 512 * j:512 * j + w],
                            start=False, stop=last,
                        )
                    if filler and tki % 2 == 1:
                        filler.pop(0)()
                # normalize: y = out / denom. High priority: this chain
                # releases the pv PSUM banks the next pair's sink needs.
                with tc.high_priority(offset=300):
                    for j in range(2):
                        pb = 64 * j
                        # copy denom to SBUF first: reciprocal_approx_fast's
                        # bitwise seed trick is wrong on raw PSUM (e10m23) reads
                        r0 = rp.tile([1, 512], f32, tag="r0", name="r0")
                        nc.vector.tensor_copy(out=r0[:, :], in_=pvs[j][64:65, :])
                        r1 = rp.tile([1, 512], f32, tag="r1", name="r1")
                        nc.vector.reciprocal_approx_fast(out=r1, in_=r0[:, :])
                        rb = rp.tile([64, 512], f32, tag="rb", name="rb")
                        nc.gpsimd.partition_broadcast(rb, r1)
                        nc.vector.tensor_mul(
                            out=yT[hp][pb:pb + 64, tq0:tq0 + 512],
                            in0=pvs[j][0:64, :],
                            in1=rb,
                        )

            def outproj(tb):
                stg = ost.tile([128, C], f32, tag="ostg", name="ostg")
                for co in range(2):
                    ps = projps.tile([128, 512], f32, tag="ps", name="ops")
                    for hd in range(2):
                        nc.tensor.matmul(
                            ps[:, :],
                            yT[hd][:, 128 * tb:128 * (tb + 1)],
                            wp[hd][:, 512 * co:512 * (co + 1)],
                            start=(hd == 0),
                            stop=(hd == 1 and not with_bias_proj),
                        )
                    if with_bias_proj:
                        nc.tensor.matmul(
                            ps[:, :],
                            ones_r[:, 0:128],
                            bp_r[:, 512 * co:512 * (co + 1)],
                            start=False, stop=True,
                        )
                    nc.any.tensor_copy(
                        out=stg[:, 512 * co:512 * (co + 1)], in_=ps[:, :]
                    )
                nc.sync.dma_start(out=out[128 * tb:128 * (tb + 1), :], in_=stg[:, :])

            # ---- program ------------------------------------------------
            # lead-in: q01/k01 all blocks, q23/k23/v for round 0
            for tqi in range(NTQ):
                proj_qk(0, tqi)
                proj_qk(2, tqi)
            proj_qk(1, 0)
            proj_qk(3, 0)
            for tb in range(4):
                proj_v(tb)
            # rounds: both attention pairs per query block; independent
            # TensorE work (next round's v/q23/k23 proj, previous round's
            # out-projection) is drained between chunk units as filler so
            # the PE never idles while ScalarE (exp) catches up.
            for tqi in range(NTQ):
                filler = []
                if tqi > 0:
                    filler += [(lambda tb=tb: outproj(tb))
                               for tb in range(4 * tqi - 4, 4 * tqi)]
                if tqi + 1 < NTQ:
                    filler += [(lambda tb=tb: proj_v(tb))
                               for tb in range(4 * tqi + 4, 4 * tqi + 8)]
                    filler.append(lambda t=tqi + 1: proj_qk(1, t))
                    filler.append(lambda t=tqi + 1: proj_qk(3, t))
                attention(0, tqi, filler)
                attention(1, tqi, filler)
                for f in filler:
                    f()
            # tail: last block's out-projection
            for tb in range(T // 128 - 4, T // 128):
                outproj(tb)

    nc.finalize()
    return nc


def make_core_inputs(x, W_qkv, b_qkv, W_proj, b_proj, sink_logit):
    """Host-side sharding: per-core input dicts (host does the transposes)."""
    import ml_dtypes
    bf16 = ml_dtypes.bfloat16

    x = np.asarray(x, dtype=np.float32)
    W_qkv = np.asarray(W_qkv, dtype=np.float32)
    b_qkv = np.asarray(b_qkv, dtype=np.float32)
    W_proj = np.asarray(W_proj, dtype=np.float32)
    b_proj = np.asarray(b_proj, dtype=np.float32)
    sink_logit = np.asarray(sink_logit, dtype=np.float32)

    xTs = [np.ascontiguousarray(x[b].T.astype(bf16)) for b in range(B)]
    e65 = np.zeros((1, 65), dtype=np.float32)
    e65[0, 64] = 1.0
    e65 = e65.astype(bf16)

    in_maps = []
    for c in range(NCORES):
        b, g = divmod(c, 4)
        h0 = HLOC * g
        q_rows = slice(GQ * g, GQ * (g + 1))
        k_rows = slice(C + GQ * g, C + GQ * (g + 1))
        v_rows = slice(2 * C + GQ * g, 2 * C + GQ * (g + 1))
        w_slice = np.concatenate(
            [W_qkv[q_rows], W_qkv[k_rows], W_qkv[v_rows]], axis=0
        )  # (768, 1024)
        b_slice = np.concatenate(
            [b_qkv[q_rows], b_qkv[k_rows], b_qkv[v_rows]], axis=0
        )  # (768,)
        sink = np.repeat(
            np.exp(sink_logit[h0:h0 + HLOC]).astype(np.float32)[:, None], 512, axis=1
        ).reshape(1, HLOC * 512)
        in_maps.append({
            "xT": xTs[b],
            "wqkvT": np.ascontiguousarray(w_slice.T).astype(bf16),
            "bqkv": b_slice[None, :].astype(bf16),
            "wpT": np.ascontiguousarray(W_proj[:, q_rows].T).astype(bf16),
            "bp": (b_proj if g == 0 else np.zeros_like(b_proj))[None, :].astype(bf16),
            "sinkrow": sink.astype(bf16),
            "e65": e65,
        })
    return in_maps


_NC_CACHE = {}


def kernel(x, W_qkv, b_qkv, W_proj, b_proj, sink_logit, _trace=False):
    from concourse.bass_utils import run_bass_kernel_spmd  # noqa: F401 (path set below)

    in_maps = make_core_inputs(x, W_qkv, b_qkv, W_proj, b_proj, sink_logit)
    with_bias_qkv = bool(np.any(np.asarray(b_qkv)))
    with_bias_proj = bool(np.any(np.asarray(b_proj)))
    key = ("bfloat16", with_bias_qkv, with_bias_proj)
    if key not in _NC_CACHE:
        _NC_CACHE[key] = build_nc("bfloat16", with_bias_qkv, with_bias_proj)
    nc = _NC_CACHE[key]

    from concourse.bass_utils import run_bass_kernel_spmd
    res = run_bass_kernel_spmd(nc, in_maps, core_ids=list(range(NCORES)), trace=_trace)

    outs = [res.results[c]["out"] for c in range(NCORES)]
    y = np.empty((B, T, C), dtype=np.float32)
    for b in range(B):
        y[b] = outs[4 * b] + outs[4 * b + 1] + outs[4 * b + 2] + outs[4 * b + 3]
    if _trace:
        return y, res
    return y


# make bass importable at module load so `from kernel import kernel` works
_import_bass()


# revision 34
# speedup vs baseline: 1.1436x; 1.1436x over previous
"""Causal self-attention with sink, sharded over 8 TRN2 NeuronCores.

Sharding: batch x head-group. Core c handles batch b=c//4 and heads
[4*(c%4), 4*(c%4)+4). Each core computes its QKV projection slice,
attention for its 4 heads, and a partial output projection; the host sums
the 4 partials per batch.

v2 layout (all matmuls bf16, fp32 PSUM accumulation):
  - xT   [C=1024, T=2048]   (host pre-transposed x[b], bf16)
  - qT/kT in SBUF as head-pair tiles [128, T] (2 heads x 64 stacked)
  - v1   [128, 16, 4, 65]   v natural [t, d] per tk-chunk/head + ones col
                            (65th) that accumulates the softmax denominator
                            inside the PV matmul
  - S^T = K^T Q for BOTH heads of a pair per (tq-block, tk-chunk) into one
    2-bank PSUM tile [128, 1024]; the two matmuls land on PE row-tiles
    (0,0)/(64,0) and run concurrently. One exp over the 1024-wide tile.
  - PV:  out^T[d, tq] (+ denom row) accumulated in PSUM per head over
    tk-chunks; sink enters the denominator via a rank-1 matmul.
  - normalize via reciprocal_approx_fast + gpsimd partition_broadcast + mul
  - out projection produces natural [t, co] partials via yT-as-stationary
  - program order interleaves v/q23/k23 projections into pair-0 attention
    and the output projection into pair-1 attention so TensorE has dense
    work while ScalarE (exp) catches up, and vice versa.
"""

import os
import sys

import numpy as np

B, T, C = 2, 2048, 1024
H, D = 16, 64
NCORES = 8
HLOC = 4           # heads per core
GQ = HLOC * D      # 256 per-core q (or k or v) features
F = 3 * GQ         # 768 per-core qkv features
NCC = C // 128     # 8 contraction chunks
NTQ = T // 512     # 4 query blocks
NTK = T // 128     # 16 key chunks
SCALE = 1.0 / np.sqrt(D)

_BASS_PATHS = ("/opt/trn_rl_repo", "/root/.axon_site/_ro/trn_rl_repo")


def _import_bass():
    for p in _BASS_PATHS:
        if os.path.isdir(p) and p not in sys.path:
            sys.path.insert(0, p)
    import concourse.bass as bass
    import concourse.mybir as mybir
    import concourse.tile as tile
    from concourse import bacc
    return bass, mybir, tile, bacc


def build_nc(mm_dt="bfloat16", with_bias_qkv=True, with_bias_proj=True,
             exp_merge=True):
    """Build the per-core Bass program (same program for all 8 cores)."""
    bass, mybir, tile, bacc = _import_bass()
    f32 = mybir.dt.float32
    mdt = getattr(mybir.dt, mm_dt)
    AF = mybir.ActivationFunctionType

    nc = bacc.Bacc("TRN2", target_bir_lowering=False, debug=False)

    xT = nc.dram_tensor("xT", [C, T], mdt, kind="ExternalInput")
    wqkvT = nc.dram_tensor("wqkvT", [C, F], mdt, kind="ExternalInput")
    bqkv = nc.dram_tensor("bqkv", [1, F], mdt, kind="ExternalInput")
    wpT = nc.dram_tensor("wpT", [GQ, C], mdt, kind="ExternalInput")
    bp = nc.dram_tensor("bp", [1, C], mdt, kind="ExternalInput")
    sinkrow = nc.dram_tensor("sinkrow", [1, HLOC * 512], mdt, kind="ExternalInput")
    e65 = nc.dram_tensor("e65", [1, 65], mdt, kind="ExternalInput")
    out = nc.dram_tensor("out", [T, C], f32, kind="ExternalOutput")

    with tile.TileContext(nc) as tc:
        with (
            tc.tile_pool(name="const", bufs=1) as const,
            tc.tile_pool(name="persist", bufs=1) as persist,
            tc.tile_pool(name="xw", bufs=1) as xw,
            tc.tile_pool(name="projps", bufs=2, space="PSUM") as projps,
            tc.tile_pool(name="s2p", bufs=2, space="PSUM") as s2p,
            tc.tile_pool(name="pvp", bufs=2, space="PSUM") as pvp,
            tc.tile_pool(name="ep", bufs=6) as ep,
            tc.tile_pool(name="rp", bufs=8) as rp,
            tc.tile_pool(name="ostage", bufs=2) as ost,
        ):
            # ---- constants ----------------------------------------------
            ones_f = const.tile([1, 512], f32, tag="ones_f")
            nc.vector.memset(ones_f, 1.0)
            ones_r = const.tile([1, 512], mdt, tag="ones")
            nc.vector.tensor_copy(out=ones_r[:], in_=ones_f[:])
            ones_col = const.tile([128, 1], f32, tag="ones_col")
            nc.vector.memset(ones_col, 1.0)
            e65_r = const.tile([1, 65], mdt, tag="e65")
            bqkv_r = const.tile([1, F], mdt, tag="bqkv")
            bp_r = const.tile([1, C], mdt, tag="bp")
            sink_r = const.tile([1, HLOC * 512], mdt, tag="sink")
            # additive causal mask for the 128x128 diagonal blocks of S^T
            # (tk on partitions, tq on free): keep where tq >= tk.
            tri = const.tile([128, 128], f32, tag="tri")
            nc.gpsimd.memset(tri, 0.0)
            nc.gpsimd.affine_select(
                out=tri, in_=tri,
                compare_op=mybir.AluOpType.is_ge,
                fill=-1e30,
                base=0,
                pattern=[[1, 128]],
                channel_multiplier=-1,
            )

            # ---- persistent activations ---------------------------------
            # qk feature-block tiles: [q01, q23, k01, k23] each [128, T]
            qk = [persist.tile([128, T], mdt, tag=f"qk{i}", name=f"qk{i}") for i in range(4)]
            # v natural + ones column
            v1 = persist.tile([128, NTK, HLOC, 65], mdt, tag="v1", name="v1")
            nc.vector.tensor_copy(
                out=v1[:, :, :, 64:65],
                in_=ones_col[:, :].to_broadcast([128, NTK, HLOC, 1]),
            )
            # normalized attention output, head pairs stacked: yT[hp] [128, T]
            yT = [persist.tile([128, T], mdt, tag=f"yT{i}", name=f"yT{i}") for i in range(2)]

            # ---- weight/x loads -----------------------------------------
            # input loads: ~0.65us serial issue cost per dma_start on the
            # Sync queue; order so the first projection group's accumulation
            # chain (cc=0..7 over x block 0) unblocks as early as possible
            wq = [xw.tile([128, F], mdt, tag=f"wqkv{i}", name=f"wqkv{i}")
                  for i in range(NCC)]
            xt = [xw.tile([128, T], mdt, tag=f"xt{i}", name=f"xt{i}")
                  for i in range(NCC)]
            for i in range(NCC):
                nc.sync.dma_start(out=wq[i][:],
                                  in_=wqkvT[128 * i:128 * (i + 1), :])
                nc.sync.dma_start(
                    out=xt[i][:, 0:512], in_=xT[128 * i:128 * (i + 1), 0:512])
            # round-0 attention constants next, then the remaining x blocks
            nc.sync.dma_start(out=e65_r[:], in_=e65[:, :])
            nc.sync.dma_start(out=sink_r[:], in_=sinkrow[:, :])
            if with_bias_qkv:
                nc.sync.dma_start(out=bqkv_r[:], in_=bqkv[:, :])
            if with_bias_proj:
                nc.sync.dma_start(out=bp_r[:], in_=bp[:, :])
            for tqi in range(1, NTQ):
                for i in range(NCC):
                    nc.sync.dma_start(
                        out=xt[i][:, 512 * tqi:512 * (tqi + 1)],
                        in_=xT[128 * i:128 * (i + 1), 512 * tqi:512 * (tqi + 1)])
            wp = []
            for i in range(GQ // 128):
                t = xw.tile([128, C], mdt, tag=f"wp{i}", name=f"wp{i}")
                nc.sync.dma_start(out=t[:], in_=wpT[128 * i:128 * (i + 1), :])
                wp.append(t)

            # ---- emission helpers ---------------------------------------
            def proj_qk(fb, tqi):
                # feature blocks in wqkvT cols: q:[0,256) k:[256,512)
                # fb: 0=q01, 1=q23, 2=k01, 3=k23 -> qk[fb] directly
                col0 = [0, 128, 256, 384][fb]
                ps = projps.tile([128, 512], f32, tag="ps", name="ps")
                for cc in range(NCC):
                    nc.tensor.matmul(
                        ps[:, :],
                        wq[cc][:, col0:col0 + 128],
                        xt[cc][:, 512 * tqi:512 * (tqi + 1)],
                        start=(cc == 0),
                        stop=(cc == NCC - 1 and not with_bias_qkv),
                    )
                if with_bias_qkv:
                    nc.tensor.matmul(
                        ps[:, :],
                        bqkv_r[:, col0:col0 + 128],
                        ones_r[:, 0:512],
                        start=False, stop=True,
                    )
                nc.vector.tensor_copy(
                    out=qk[fb][:, 512 * tqi:512 * (tqi + 1)], in_=ps[:, :]
                )

            def proj_v(tb):
                ps = projps.tile([128, GQ], f32, tag="ps", name="psv")
                for cc in range(NCC):
                    nc.tensor.matmul(
                        ps[:, :],
                        xt[cc][:, 128 * tb:128 * (tb + 1)],
                        wq[cc][:, 512:768],
                        start=(cc == 0),
                        stop=(cc == NCC - 1 and not with_bias_qkv),
                    )
                if with_bias_qkv:
                    nc.tensor.matmul(
                        ps[:, :],
                        ones_r[:, 0:128],
                        bqkv_r[:, 512:768],
                        start=False, stop=True,
                    )
                nc.vector.tensor_copy(
                    out=v1[:, tb, :, 0:64],
                    in_=ps[:].rearrange("p (h d) -> p h d", h=HLOC),
                )

            def attention(hp, tqi, filler=None):
                # both heads of pair hp for query block tqi; `filler` is a
                # list of closures emitting independent TensorE work, drained
                # between chunk units so the PE never starves on exp latency.
                tq0 = 512 * tqi
                q_t, k_t = qk[hp], qk[2 + hp]
                pvs = None
                nchunks = tq0 // 128 + 4
                for tki in range(nchunks):
                    tk0 = 128 * tki
                    last = tki == nchunks - 1
                    full = tk0 < tq0
                    m = 0 if full else (tk0 - tq0) // 128
                    w = 512 - 128 * m
                    s2 = s2p.tile([128, 1024], f32, tag="s2", name="s2")
                    # S^T for both heads; stationary k at partition bases 0/64
                    # -> PE row-tiles (0,0)/(64,0), concurrent execution.
                    for j in range(2):
                        pb = 64 * j
                        nc.tensor.matmul(
                            s2[:, 512 * j:512 * j + w],
                            k_t[pb:pb + 64, tk0:tk0 + 128],
                            q_t[pb:pb + 64, tq0 + 128 * m:tq0 + 512],
                            start=True, stop=True,
                        )
                    if pvs is None:
                        # sink -> denominator row (also zero-fills rows
                        # 0..63). Emitted after the first S-pair so a wait
                        # on the pv bank (prev pair's normalize) doesn't
                        # head-block the ready S work in the in-order queue.
                        pvs = []
                        for j in range(2):
                            h = 2 * hp + j
                            pv = pvp.tile([65, 512], f32, tag="pv", name="pv")
                            nc.tensor.matmul(
                                pv[:, :], e65_r[:, :],
                                sink_r[0:1, h * 512:(h + 1) * 512],
                                start=True, stop=False,
                            )
                            pvs.append(pv)
                    e = ep.tile([128, 1024], mdt, tag="e", name="e")
                    if not exp_merge:
                        for j in range(2):
                            nc.scalar.activation(
                                out=e[:, 512 * j:512 * j + w],
                                in_=s2[:, 512 * j:512 * j + w],
                                func=AF.Exp, scale=SCALE)
                    elif full:
                        nc.scalar.activation(out=e[:, :], in_=s2[:, :],
                                             func=AF.Exp, scale=SCALE)
                    else:
                        # single ACT over the valid strided region of both heads
                        nc.scalar.activation(
                            out=e.rearrange("p (j q) -> p j q", j=2)[:, :, 0:w],
                            in_=s2.rearrange("p (j q) -> p j q", j=2)[:, :, 0:w],
                            func=AF.Exp, scale=SCALE)
                    if not full:
                        # causal mask: zero the upper triangle of the first
                        # visible 128 cols of e for both heads (idle GpSimd;
                        # keeps the DVE off the exp->PV critical path)
                        ev = e.rearrange("p (j q) -> p j q", j=2)[:, :, 0:128]
                        nc.gpsimd.affine_select(
                            out=ev, in_=ev,
                            compare_op=mybir.AluOpType.is_ge,
                            fill=0.0,
                            base=0,
                            pattern=[[0, 2], [1, 128]],
                            channel_multiplier=-1,
                        )
                    for j in range(2):
                        h = 2 * hp + j
                        nc.tensor.matmul(
                            pvs[j][:, 128 * m:512],
                            v1[:, tki, h, :],
                            e[:, 512 * j:512 * j + w],
                            start=False, stop=last,
                        )
                    if filler and tki % 2 == 1:
                        filler.pop(0)()
                # normalize: y = out / denom. High priority: this chain
                # releases the pv PSUM banks the next pair's sink needs.
                with tc.high_priority(offset=300):
                    for j in range(2):
                        pb = 64 * j
                        # copy denom to SBUF first: reciprocal_approx_fast's
                        # bitwise seed trick is wrong on raw PSUM (e10m23) reads
                        r0 = rp.tile([1, 512], f32, tag="r0", name="r0")
                        nc.vector.tensor_copy(out=r0[:, :], in_=pvs[j][64:65, :])
                        r1 = rp.tile([1, 512], f32, tag="r1", name="r1")
                        nc.vector.reciprocal_approx_fast(out=r1, in_=r0[:, :])
                        rb = rp.tile([64, 512], f32, tag="rb", name="rb")
                        nc.gpsimd.partition_broadcast(rb, r1)
                        nc.vector.tensor_mul(
                            out=yT[hp][pb:pb + 64, tq0:tq0 + 512],
                            in0=pvs[j][0:64, :],
                            in1=rb,
                        )

            def outproj(tb):
                stg = ost.tile([128, C], f32, tag="ostg", name="ostg")
                for co in range(2):
                    ps = projps.tile([128, 512], f32, tag="ps", name="ops")
                    for hd in range(2):
                        nc.tensor.matmul(
                            ps[:, :],
                            yT[hd][:, 128 * tb:128 * (tb + 1)],
                            wp[hd][:, 512 * co:512 * (co + 1)],
                            start=(hd == 0),
                            stop=(hd == 1 and not with_bias_proj),
                        )
                    if with_bias_proj:
                        nc.tensor.matmul(
                            ps[:, :],
                            ones_r[:, 0:128],
                            bp_r[:, 512 * co:512 * (co + 1)],
                            start=False, stop=True,
                        )
                    nc.any.tensor_copy(
                        out=stg[:, 512 * co:512 * (co + 1)], in_=ps[:, :]
                    )
                nc.sync.dma_start(out=out[128 * tb:128 * (tb + 1), :], in_=stg[:, :])

            # ---- program ------------------------------------------------
            # lead-in: q01/k01 all blocks, q23/k23/v for round 0
            for tqi in range(NTQ):
                proj_qk(0, tqi)
                proj_qk(2, tqi)
            proj_qk(1, 0)
            proj_qk(3, 0)
            for tb in range(4):
                proj_v(tb)
            # rounds: both attention pairs per query block; independent
            # TensorE work (next round's v/q23/k23 proj, previous round's
            # out-projection) is drained between chunk units as filler so
            # the PE never idles while ScalarE (exp) catches up.
            for tqi in range(NTQ):
                filler = []
                if tqi > 0:
                    filler += [(lambda tb=tb: outproj(tb))
                               for tb in range(4 * tqi - 4, 4 * tqi)]
                if tqi + 1 < NTQ:
                    filler += [(lambda tb=tb: proj_v(tb))
                               for tb in range(4 * tqi + 4, 4 * tqi + 8)]
                    filler.append(lambda t=tqi + 1: proj_qk(1, t))
                    filler.append(lambda t=tqi + 1: proj_qk(3, t))
                attention(0, tqi, filler)
                attention(1, tqi, filler)
                for f in filler:
                    f()
            # tail: last block's out-projection
            for tb in range(T // 128 - 4, T // 128):
                outproj(tb)

    nc.finalize()
    return nc


def make_core_inputs(x, W_qkv, b_qkv, W_proj, b_proj, sink_logit):
    """Host-side sharding: per-core input dicts (host does the transposes)."""
    import ml_dtypes
    bf16 = ml_dtypes.bfloat16

    x = np.asarray(x, dtype=np.float32)
    W_qkv = np.asarray(W_qkv, dtype=np.float32)
    b_qkv = np.asarray(b_qkv, dtype=np.float32)
    W_proj = np.asarray(W_proj, dtype=np.float32)
    b_proj = np.asarray(b_proj, dtype=np.float32)
    sink_logit = np.asarray(sink_logit, dtype=np.float32)

    xTs = [np.ascontiguousarray(x[b].T.astype(bf16)) for b in range(B)]
    e65 = np.zeros((1, 65), dtype=np.float32)
    e65[0, 64] = 1.0
    e65 = e65.astype(bf16)

    in_maps = []
    for c in range(NCORES):
        b, g = divmod(c, 4)
        h0 = HLOC * g
        q_rows = slice(GQ * g, GQ * (g + 1))
        k_rows = slice(C + GQ * g, C + GQ * (g + 1))
        v_rows = slice(2 * C + GQ * g, 2 * C + GQ * (g + 1))
        w_slice = np.concatenate(
            [W_qkv[q_rows], W_qkv[k_rows], W_qkv[v_rows]], axis=0
        )  # (768, 1024)
        b_slice = np.concatenate(
            [b_qkv[q_rows], b_qkv[k_rows], b_qkv[v_rows]], axis=0
        )  # (768,)
        sink = np.repeat(
            np.exp(sink_logit[h0:h0 + HLOC]).astype(np.float32)[:, None], 512, axis=1
        ).reshape(1, HLOC * 512)
        in_maps.append({
            "xT": xTs[b],
            "wqkvT": np.ascontiguousarray(w_slice.T).astype(bf16),
            "bqkv": b_slice[None, :].astype(bf16),
            "wpT": np.ascontiguousarray(W_proj[:, q_rows].T).astype(bf16),
            "bp": (b_proj if g == 0 else np.zeros_like(b_proj))[None, :].astype(bf16),
            "sinkrow": sink.astype(bf16),
            "e65": e65,
        })
    return in_maps


_NC_CACHE = {}


def kernel(x, W_qkv, b_qkv, W_proj, b_proj, sink_logit, _trace=False):
    from concourse.bass_utils import run_bass_kernel_spmd  # noqa: F401 (path set below)

    in_maps = make_core_inputs(x, W_qkv, b_qkv, W_proj, b_proj, sink_logit)
    with_bias_qkv = bool(np.any(np.asarray(b_qkv)))
    with_bias_proj = bool(np.any(np.asarray(b_proj)))
    key = ("bfloat16", with_bias_qkv, with_bias_proj)
    if key not in _NC_CACHE:
        _NC_CACHE[key] = build_nc("bfloat16", with_bias_qkv, with_bias_proj)
    nc = _NC_CACHE[key]

    from concourse.bass_utils import run_bass_kernel_spmd
    res = run_bass_kernel_spmd(nc, in_maps, core_ids=list(range(NCORES)), trace=_trace)

    outs = [res.results[c]["out"] for c in range(NCORES)]
    y = np.empty((B, T, C), dtype=np.float32)
    for b in range(B):
        y[b] = outs[4 * b] + outs[4 * b + 1] + outs[4 * b + 2] + outs[4 * b + 3]
    if _trace:
        return y, res
    return y


# make bass importable at module load so `from kernel import kernel` works
_import_bass()


# revision 37
# speedup vs baseline: 1.2184x; 1.0654x over previous
"""Causal self-attention with sink, sharded over 8 TRN2 NeuronCores.

Sharding: batch x head-group. Core c handles batch b=c//4 and heads
[4*(c%4), 4*(c%4)+4). Each core computes its QKV projection slice,
attention for its 4 heads, and a partial output projection; the host sums
the 4 partials per batch.

v2 layout (all matmuls bf16, fp32 PSUM accumulation):
  - xT   [C=1024, T=2048]   (host pre-transposed x[b], bf16)
  - qT/kT in SBUF as head-pair tiles [128, T] (2 heads x 64 stacked)
  - v1   [128, 16, 4, 65]   v natural [t, d] per tk-chunk/head + ones col
                            (65th) that accumulates the softmax denominator
                            inside the PV matmul
  - S^T = K^T Q for BOTH heads of a pair per (tq-block, tk-chunk) into one
    2-bank PSUM tile [128, 1024]; the two matmuls land on PE row-tiles
    (0,0)/(64,0) and run concurrently. One exp over the 1024-wide tile.
  - PV:  out^T[d, tq] (+ denom row) accumulated in PSUM per head over
    tk-chunks; sink enters the denominator via a rank-1 matmul.
  - normalize via reciprocal_approx_fast + gpsimd partition_broadcast + mul
  - out projection produces natural [t, co] partials via yT-as-stationary
  - program order interleaves v/q23/k23 projections into pair-0 attention
    and the output projection into pair-1 attention so TensorE has dense
    work while ScalarE (exp) catches up, and vice versa.
"""

import os
import sys

import numpy as np

B, T, C = 2, 2048, 1024
H, D = 16, 64
NCORES = 8
HLOC = 4           # heads per core
GQ = HLOC * D      # 256 per-core q (or k or v) features
F = 3 * GQ         # 768 per-core qkv features
NCC = C // 128     # 8 contraction chunks
NTQ = T // 512     # 4 query blocks
NTK = T // 128     # 16 key chunks
SCALE = 1.0 / np.sqrt(D)

_BASS_PATHS = ("/opt/trn_rl_repo", "/root/.axon_site/_ro/trn_rl_repo")


def _import_bass():
    for p in _BASS_PATHS:
        if os.path.isdir(p) and p not in sys.path:
            sys.path.insert(0, p)
    import concourse.bass as bass
    import concourse.mybir as mybir
    import concourse.tile as tile
    from concourse import bacc
    return bass, mybir, tile, bacc


def build_nc(mm_dt="bfloat16", with_bias_qkv=True, with_bias_proj=True,
             exp_merge=True):
    """Build the per-core Bass program (same program for all 8 cores)."""
    bass, mybir, tile, bacc = _import_bass()
    f32 = mybir.dt.float32
    mdt = getattr(mybir.dt, mm_dt)
    AF = mybir.ActivationFunctionType

    nc = bacc.Bacc("TRN2", target_bir_lowering=False, debug=False)

    xT = nc.dram_tensor("xT", [C, T], mdt, kind="ExternalInput")
    wqkvT = nc.dram_tensor("wqkvT", [C, F], mdt, kind="ExternalInput")
    bqkv = nc.dram_tensor("bqkv", [1, F], mdt, kind="ExternalInput")
    wpT = nc.dram_tensor("wpT", [GQ, C], mdt, kind="ExternalInput")
    bp = nc.dram_tensor("bp", [1, C], mdt, kind="ExternalInput")
    sinkrow = nc.dram_tensor("sinkrow", [1, HLOC * 512], mdt, kind="ExternalInput")
    e65 = nc.dram_tensor("e65", [1, 65], mdt, kind="ExternalInput")
    out = nc.dram_tensor("out", [T, C], f32, kind="ExternalOutput")

    with tile.TileContext(nc) as tc:
        with (
            tc.tile_pool(name="const", bufs=1) as const,
            tc.tile_pool(name="persist", bufs=1) as persist,
            tc.tile_pool(name="xw", bufs=1) as xw,
            tc.tile_pool(name="projps", bufs=2, space="PSUM") as projps,
            tc.tile_pool(name="s2p", bufs=2, space="PSUM") as s2p,
            tc.tile_pool(name="pvp", bufs=2, space="PSUM") as pvp,
            tc.tile_pool(name="ep", bufs=6) as ep,
            tc.tile_pool(name="rp", bufs=8) as rp,
            tc.tile_pool(name="ostage", bufs=2) as ost,
        ):
            # ---- constants ----------------------------------------------
            ones_f = const.tile([1, 512], f32, tag="ones_f")
            nc.vector.memset(ones_f, 1.0)
            ones_r = const.tile([1, 512], mdt, tag="ones")
            nc.vector.tensor_copy(out=ones_r[:], in_=ones_f[:])
            ones_col = const.tile([128, 1], f32, tag="ones_col")
            nc.vector.memset(ones_col, 1.0)
            e65_r = const.tile([1, 65], mdt, tag="e65")
            bqkv_r = const.tile([1, F], mdt, tag="bqkv")
            bp_r = const.tile([1, C], mdt, tag="bp")
            sink_r = const.tile([1, HLOC * 512], mdt, tag="sink")
            # additive causal mask for the 128x128 diagonal blocks of S^T
            # (tk on partitions, tq on free): keep where tq >= tk.
            tri = const.tile([128, 128], f32, tag="tri")
            nc.gpsimd.memset(tri, 0.0)
            nc.gpsimd.affine_select(
                out=tri, in_=tri,
                compare_op=mybir.AluOpType.is_ge,
                fill=-1e30,
                base=0,
                pattern=[[1, 128]],
                channel_multiplier=-1,
            )

            # ---- persistent activations ---------------------------------
            # qk feature-block tiles: [q01, q23, k01, k23] each [128, T]
            qk = [persist.tile([128, T], mdt, tag=f"qk{i}", name=f"qk{i}") for i in range(4)]
            # v natural + ones column
            v1 = persist.tile([128, NTK, HLOC, 65], mdt, tag="v1", name="v1")
            nc.vector.tensor_copy(
                out=v1[:, :, :, 64:65],
                in_=ones_col[:, :].to_broadcast([128, NTK, HLOC, 1]),
            )
            # normalized attention output, head pairs stacked: yT[hp] [128, T]
            yT = [persist.tile([128, T], mdt, tag=f"yT{i}", name=f"yT{i}") for i in range(2)]

            # ---- weight/x loads -----------------------------------------
            # input loads: ~0.65us serial issue cost per dma_start on the
            # Sync queue; order so the first projection group's accumulation
            # chain (cc=0..7 over x block 0) unblocks as early as possible
            wq = [xw.tile([128, F], mdt, tag=f"wqkv{i}", name=f"wqkv{i}")
                  for i in range(NCC)]
            xt = [xw.tile([128, T], mdt, tag=f"xt{i}", name=f"xt{i}")
                  for i in range(NCC)]
            # parallel issue: weights on the Sync DMA queue, x block 0 on the
            # (idle until attention) Scalar DMA queue
            for i in range(NCC):
                nc.sync.dma_start(out=wq[i][:],
                                  in_=wqkvT[128 * i:128 * (i + 1), :])
                nc.scalar.dma_start(
                    out=xt[i][:, 0:512], in_=xT[128 * i:128 * (i + 1), 0:512])
            # round-0 attention constants next, then the remaining x blocks
            nc.scalar.dma_start(out=e65_r[:], in_=e65[:, :])
            nc.scalar.dma_start(out=sink_r[:], in_=sinkrow[:, :])
            if with_bias_qkv:
                nc.sync.dma_start(out=bqkv_r[:], in_=bqkv[:, :])
            if with_bias_proj:
                nc.sync.dma_start(out=bp_r[:], in_=bp[:, :])
            for tqi in range(1, NTQ):
                for i in range(NCC):
                    nc.sync.dma_start(
                        out=xt[i][:, 512 * tqi:512 * (tqi + 1)],
                        in_=xT[128 * i:128 * (i + 1), 512 * tqi:512 * (tqi + 1)])
            wp = []
            for i in range(GQ // 128):
                t = xw.tile([128, C], mdt, tag=f"wp{i}", name=f"wp{i}")
                nc.sync.dma_start(out=t[:], in_=wpT[128 * i:128 * (i + 1), :])
                wp.append(t)

            # ---- emission helpers ---------------------------------------
            def proj_qk(fb, tqi):
                # feature blocks in wqkvT cols: q:[0,256) k:[256,512)
                # fb: 0=q01, 1=q23, 2=k01, 3=k23 -> qk[fb] directly
                col0 = [0, 128, 256, 384][fb]
                ps = projps.tile([128, 512], f32, tag="ps", name="ps")
                for cc in range(NCC):
                    nc.tensor.matmul(
                        ps[:, :],
                        wq[cc][:, col0:col0 + 128],
                        xt[cc][:, 512 * tqi:512 * (tqi + 1)],
                        start=(cc == 0),
                        stop=(cc == NCC - 1 and not with_bias_qkv),
                    )
                if with_bias_qkv:
                    nc.tensor.matmul(
                        ps[:, :],
                        bqkv_r[:, col0:col0 + 128],
                        ones_r[:, 0:512],
                        start=False, stop=True,
                    )
                nc.vector.tensor_copy(
                    out=qk[fb][:, 512 * tqi:512 * (tqi + 1)], in_=ps[:, :]
                )

            def proj_v(tb):
                ps = projps.tile([128, GQ], f32, tag="ps", name="psv")
                for cc in range(NCC):
                    nc.tensor.matmul(
                        ps[:, :],
                        xt[cc][:, 128 * tb:128 * (tb + 1)],
                        wq[cc][:, 512:768],
                        start=(cc == 0),
                        stop=(cc == NCC - 1 and not with_bias_qkv),
                    )
                if with_bias_qkv:
                    nc.tensor.matmul(
                        ps[:, :],
                        ones_r[:, 0:128],
                        bqkv_r[:, 512:768],
                        start=False, stop=True,
                    )
                nc.vector.tensor_copy(
                    out=v1[:, tb, :, 0:64],
                    in_=ps[:].rearrange("p (h d) -> p h d", h=HLOC),
                )

            def attention(hp, tqi, filler=None):
                # both heads of pair hp for query block tqi; `filler` is a
                # list of closures emitting independent TensorE work, drained
                # between chunk units so the PE never starves on exp latency.
                tq0 = 512 * tqi
                q_t, k_t = qk[hp], qk[2 + hp]
                pvs = None
                nchunks = tq0 // 128 + 4
                for tki in range(nchunks):
                    tk0 = 128 * tki
                    last = tki == nchunks - 1
                    full = tk0 < tq0
                    m = 0 if full else (tk0 - tq0) // 128
                    w = 512 - 128 * m
                    s2 = s2p.tile([128, 1024], f32, tag="s2", name="s2")
                    # S^T for both heads; stationary k at partition bases 0/64
                    # -> PE row-tiles (0,0)/(64,0), concurrent execution.
                    for j in range(2):
                        pb = 64 * j
                        nc.tensor.matmul(
                            s2[:, 512 * j:512 * j + w],
                            k_t[pb:pb + 64, tk0:tk0 + 128],
                            q_t[pb:pb + 64, tq0 + 128 * m:tq0 + 512],
                            start=True, stop=True,
                        )
                    if pvs is None:
                        # sink -> denominator row (also zero-fills rows
                        # 0..63). Emitted after the first S-pair so a wait
                        # on the pv bank (prev pair's normalize) doesn't
                        # head-block the ready S work in the in-order queue.
                        pvs = []
                        for j in range(2):
                            h = 2 * hp + j
                            pv = pvp.tile([65, 512], f32, tag="pv", name="pv")
                            nc.tensor.matmul(
                                pv[:, :], e65_r[:, :],
                                sink_r[0:1, h * 512:(h + 1) * 512],
                                start=True, stop=False,
                            )
                            pvs.append(pv)
                    e = ep.tile([128, 1024], mdt, tag="e", name="e")
                    if not exp_merge:
                        for j in range(2):
                            nc.scalar.activation(
                                out=e[:, 512 * j:512 * j + w],
                                in_=s2[:, 512 * j:512 * j + w],
                                func=AF.Exp, scale=SCALE)
                    elif full:
                        nc.scalar.activation(out=e[:, :], in_=s2[:, :],
                                             func=AF.Exp, scale=SCALE)
                    else:
                        # single ACT over the valid strided region of both heads
                        nc.scalar.activation(
                            out=e.rearrange("p (j q) -> p j q", j=2)[:, :, 0:w],
                            in_=s2.rearrange("p (j q) -> p j q", j=2)[:, :, 0:w],
                            func=AF.Exp, scale=SCALE)
                    if not full:
                        # causal mask: zero the upper triangle of the first
                        # visible 128 cols of e for both heads (idle GpSimd;
                        # keeps the DVE off the exp->PV critical path)
                        ev = e.rearrange("p (j q) -> p j q", j=2)[:, :, 0:128]
                        nc.gpsimd.affine_select(
                            out=ev, in_=ev,
                            compare_op=mybir.AluOpType.is_ge,
                            fill=0.0,
                            base=0,
                            pattern=[[0, 2], [1, 128]],
                            channel_multiplier=-1,
                        )
                    for j in range(2):
                        h = 2 * hp + j
                        nc.tensor.matmul(
                            pvs[j][:, 128 * m:512],
                            v1[:, tki, h, :],
                            e[:, 512 * j:512 * j + w],
                            start=False, stop=last,
                        )
                    if filler and tki % 2 == 1:
                        filler.pop(0)()
                # normalize: y = out / denom. High priority: this chain
                # releases the pv PSUM banks the next pair's sink needs.
                with tc.high_priority(offset=300):
                    for j in range(2):
                        pb = 64 * j
                        # copy denom to SBUF first: reciprocal_approx_fast's
                        # bitwise seed trick is wrong on raw PSUM (e10m23) reads
                        r0 = rp.tile([1, 512], f32, tag="r0", name="r0")
                        nc.vector.tensor_copy(out=r0[:, :], in_=pvs[j][64:65, :])
                        r1 = rp.tile([1, 512], f32, tag="r1", name="r1")
                        nc.vector.reciprocal_approx_fast(out=r1, in_=r0[:, :])
                        rb = rp.tile([64, 512], f32, tag="rb", name="rb")
                        nc.gpsimd.partition_broadcast(rb, r1)
                        nc.vector.tensor_mul(
                            out=yT[hp][pb:pb + 64, tq0:tq0 + 512],
                            in0=pvs[j][0:64, :],
                            in1=rb,
                        )

            def outproj(tb):
                stg = ost.tile([128, C], f32, tag="ostg", name="ostg")
                for co in range(2):
                    ps = projps.tile([128, 512], f32, tag="ps", name="ops")
                    for hd in range(2):
                        nc.tensor.matmul(
                            ps[:, :],
                            yT[hd][:, 128 * tb:128 * (tb + 1)],
                            wp[hd][:, 512 * co:512 * (co + 1)],
                            start=(hd == 0),
                            stop=(hd == 1 and not with_bias_proj),
                        )
                    if with_bias_proj:
                        nc.tensor.matmul(
                            ps[:, :],
                            ones_r[:, 0:128],
                            bp_r[:, 512 * co:512 * (co + 1)],
                            start=False, stop=True,
                        )
                    nc.any.tensor_copy(
                        out=stg[:, 512 * co:512 * (co + 1)], in_=ps[:, :]
                    )
                nc.sync.dma_start(out=out[128 * tb:128 * (tb + 1), :], in_=stg[:, :])

            # ---- program ------------------------------------------------
            # lead-in: q01/k01 all blocks, q23/k23/v for round 0
            for tqi in range(NTQ):
                proj_qk(0, tqi)
                proj_qk(2, tqi)
            proj_qk(1, 0)
            proj_qk(3, 0)
            for tb in range(4):
                proj_v(tb)
            # rounds: both attention pairs per query block; independent
            # TensorE work (next round's v/q23/k23 proj, previous round's
            # out-projection) is drained between chunk units as filler so
            # the PE never idles while ScalarE (exp) catches up.
            # out-projection filler weighted toward the late, exp-bound
            # rounds (round 3 has the most attention units per filler)
            OUTPROJ_ROUND = {2: range(0, 4), 3: range(4, 12)}
            for tqi in range(NTQ):
                filler = []
                for tb in OUTPROJ_ROUND.get(tqi, ()):
                    filler.append(lambda tb=tb: outproj(tb))
                if tqi + 1 < NTQ:
                    filler += [(lambda tb=tb: proj_v(tb))
                               for tb in range(4 * tqi + 4, 4 * tqi + 8)]
                    filler.append(lambda t=tqi + 1: proj_qk(1, t))
                    filler.append(lambda t=tqi + 1: proj_qk(3, t))
                attention(0, tqi, filler)
                attention(1, tqi, filler)
                for f in filler:
                    f()
            # tail: last block's out-projection
            for tb in range(12, T // 128):
                outproj(tb)

    nc.finalize()
    return nc


def make_core_inputs(x, W_qkv, b_qkv, W_proj, b_proj, sink_logit):
    """Host-side sharding: per-core input dicts (host does the transposes)."""
    import ml_dtypes
    bf16 = ml_dtypes.bfloat16

    x = np.asarray(x, dtype=np.float32)
    W_qkv = np.asarray(W_qkv, dtype=np.float32)
    b_qkv = np.asarray(b_qkv, dtype=np.float32)
    W_proj = np.asarray(W_proj, dtype=np.float32)
    b_proj = np.asarray(b_proj, dtype=np.float32)
    sink_logit = np.asarray(sink_logit, dtype=np.float32)

    xTs = [np.ascontiguousarray(x[b].T.astype(bf16)) for b in range(B)]
    e65 = np.zeros((1, 65), dtype=np.float32)
    e65[0, 64] = 1.0
    e65 = e65.astype(bf16)

    in_maps = []
    for c in range(NCORES):
        b, g = divmod(c, 4)
        h0 = HLOC * g
        q_rows = slice(GQ * g, GQ * (g + 1))
        k_rows = slice(C + GQ * g, C + GQ * (g + 1))
        v_rows = slice(2 * C + GQ * g, 2 * C + GQ * (g + 1))
        w_slice = np.concatenate(
            [W_qkv[q_rows], W_qkv[k_rows], W_qkv[v_rows]], axis=0
        )  # (768, 1024)
        b_slice = np.concatenate(
            [b_qkv[q_rows], b_qkv[k_rows], b_qkv[v_rows]], axis=0
        )  # (768,)
        sink = np.repeat(
            np.exp(sink_logit[h0:h0 + HLOC]).astype(np.float32)[:, None], 512, axis=1
        ).reshape(1, HLOC * 512)
        in_maps.append({
            "xT": xTs[b],
            "wqkvT": np.ascontiguousarray(w_slice.T).astype(bf16),
            "bqkv": b_slice[None, :].astype(bf16),
            "wpT": np.ascontiguousarray(W_proj[:, q_rows].T).astype(bf16),
            "bp": (b_proj if g == 0 else np.zeros_like(b_proj))[None, :].astype(bf16),
            "sinkrow": sink.astype(bf16),
            "e65": e65,
        })
    return in_maps


_NC_CACHE = {}


def kernel(x, W_qkv, b_qkv, W_proj, b_proj, sink_logit, _trace=False):
    from concourse.bass_utils import run_bass_kernel_spmd  # noqa: F401 (path set below)

    in_maps = make_core_inputs(x, W_qkv, b_qkv, W_proj, b_proj, sink_logit)
    with_bias_qkv = bool(np.any(np.asarray(b_qkv)))
    with_bias_proj = bool(np.any(np.asarray(b_proj)))
    key = ("bfloat16", with_bias_qkv, with_bias_proj)
    if key not in _NC_CACHE:
        _NC_CACHE[key] = build_nc("bfloat16", with_bias_qkv, with_bias_proj)
    nc = _NC_CACHE[key]

    from concourse.bass_utils import run_bass_kernel_spmd
    res = run_bass_kernel_spmd(nc, in_maps, core_ids=list(range(NCORES)), trace=_trace)

    outs = [res.results[c]["out"] for c in range(NCORES)]
    y = np.empty((B, T, C), dtype=np.float32)
    for b in range(B):
        y[b] = outs[4 * b] + outs[4 * b + 1] + outs[4 * b + 2] + outs[4 * b + 3]
    if _trace:
        return y, res
    return y


# make bass importable at module load so `from kernel import kernel` works
_import_bass()
